# revision 10
# baseline (speedup 1.0000x reference)
"""Trainium2 Bass kernel for nn_ConvHybridFilter.

Reference computes: 0.5 * mean(sigmoid(conv2d_4x4_valid(data) + b)) + 0.5 * qexp
where qexp is the expectation of a 16-qubit circuit with threshold-binarized
RX angles (theta in {0, pi}).

Math fact used here: RX(pi) = -i*X and RX(0) = I, and both X and CX only
permute computational basis states. The post-Hadamard state is the uniform
superposition, which is invariant (up to a global phase) under every gate in
the circuit, so P(q0=1) = 1/2 exactly and qexp == 0 for every sample
(verified bit-exact against the float32 jax reference: absmax(qexp) == 0.0).

So the kernel only needs the classical branch:
    out = broadcast_{B=256}( 0.5 * mean(sigmoid(conv(data, w) + b)) )

Device strategy (pure data parallel over 8 cores, 32 images each):
  - The 4x4 valid conv over a 64x64 image runs on the TensorEngine as 4
    accumulating fp32r matmuls (one per kernel column j), contracting over
    the 64 image rows with a banded Toeplitz lhsT built from conv_w.
    Two images are packed per matmul (128 = 2*64 contraction rows) using a
    block-diagonal Toeplitz, so M = 122 output partitions (2*61 rows).
  - fp32r restrictions (walrus s3d3_mm_fp32r): moving AP must be 1D
    contiguous and N < 512. So the rhs for kernel column j is a contiguous
    sliding window [j : j+445] over 7 image-pairs laid side by side at
    stride 64; output columns s*64+[61..63] accumulate junk from the pair
    boundary and are excluded from the reduction.
  - ScalarEngine applies sigmoid(x + b) on the valid 2D slice of PSUM and
    row-sums it via accum_out.  conv_b is baked in as a const-AP bias.
  - Per-core partial sums (122 x 3 chunks) are DMA'd out; the host combines
    them in float64 and broadcasts the scalar result.
  - Tail-drain sem joins: walrus allows ~1 sync wait per instruction, so SP
    nops (add_dep_helper) absorb each outstanding semaphore before Tile's
    exit drain.
"""

import numpy as np

N_CORES = 8
B = 256
PER_CORE = B // N_CORES          # 32 images per core
H = W = 64
KH = KW = 4
OH = OW = 61                     # valid conv output
NPAIR = PER_CORE // 2            # 16 image pairs per core
CHUNK_PAIRS = (7, 7, 2)          # max 7 pairs: (7-1)*64+61 = 445 <= 488 < 512
NCHUNK = len(CHUNK_PAIRS)
M = 2 * OH                       # 122 output partitions per pair
WT_COLS = KW * M                 # 488

_COMPILED = {}


XPAD = 4                         # zero padding so the j=3 window stays in bounds


def _chunk_n(pairs):
    # contiguous moving size covering all pairs, rounded up to even
    # (walrus s3d3_mm_fp32r rejects odd moving sizes)
    return (pairs - 1) * W + OH + 1


def _build_bass(conv_b_value):
    import concourse.bass as bass
    import concourse.mybir as mybir
    import concourse.tile as tile
    from concourse.tile import add_dep_helper

    f32 = mybir.dt.float32
    f32r = mybir.dt.float32r

    nc = bass.Bass()
    conv_b_value = float(conv_b_value)
    if (f32, conv_b_value) not in nc.const_aps.aps:
        bias_t = nc.alloc_sbuf_tensor("const-bias", [128, 1], f32)
        nc.gpsimd.memset(bias_t.ap(), conv_b_value)
        nc.const_aps.aps[(f32, conv_b_value)] = bias_t.ap()
        nc.all_engine_barrier()

    blob_d = []
    for c, pairs in enumerate(CHUNK_PAIRS):
        cols = pairs * W + XPAD + (WT_COLS if c == 0 else 0)
        blob_d.append(nc.declare_dram_parameter(
            f"blob{c}", [128, cols], f32r, isOutput=False))
    s_d = nc.declare_dram_parameter("sums", [M, NCHUNK], f32, isOutput=True)

    with tile.TileContext(nc) as tc:
        with (
            tc.tile_pool(name="sbuf", bufs=1) as pool,
            tc.tile_pool(name="psum", bufs=1, space="PSUM") as psum,
        ):
            bt = [
                pool.tile([128, pairs * W + XPAD + (WT_COLS if c == 0 else 0)], f32r,
                          name=f"b{c}", tag=f"b{c}")
                for c, pairs in enumerate(CHUNK_PAIRS)
            ]
            st = pool.tile([128, NCHUNK], f32, tag="st")
            acc = psum.tile([128, NCHUNK, 512], f32, tag="acc")

            dmas = [nc.sync.dma_start(bt[c][:], blob_d[c][:])
                    for c in range(NCHUNK)]

            last_mm = None
            for c, pairs in enumerate(CHUNK_PAIRS):
                n = _chunk_n(pairs)
                x_off = WT_COLS if c == 0 else 0
                for j in range(KW):
                    last_mm = nc.tensor.matmul(
                        acc[0:M, c, 0:n],
                        bt[0][:, j * M:(j + 1) * M],
                        bt[c][:, x_off + j: x_off + j + n],
                        start=(j == 0),
                        stop=(j == KW - 1),
                    )

            last_act = None
            for c, pairs in enumerate(CHUNK_PAIRS):
                valid = (acc[0:M, c, 0:pairs * W]
                         .rearrange("p (s w) -> p s w", w=W)[:, :, 0:OH])
                last_act = nc.scalar.activation(
                    valid,
                    valid,
                    mybir.ActivationFunctionType.Sigmoid,
                    bias=conv_b_value,
                    accum_out=st[0:M, c:c + 1],
                )

            # Spread the kernel-tail drain's sem waits across SP nops:
            # walrus rejects instructions carrying more than one sync wait,
            # and Tile's exit drain would otherwise wait on every proc.
            for dep in [last_mm.ins, last_act.ins] + [d.ins for d in dmas]:
                nop = nc.sync.nop(nofuse=True, hint="join")
                add_dep_helper(nop.ins, dep, reason="pre-drain sem join")

            nc.sync.dma_start(s_d[:], st[0:M, :])

    return nc


def _toeplitz(conv_w):
    """Block-diagonal banded lhsT, flattened to (128, KW*M) float32 (j-major)."""
    w = np.asarray(conv_w, dtype=np.float32).reshape(KH, KW)
    T = np.zeros((128, KW, M), dtype=np.float32)
    for i in range(KH):
        for m in range(OH):
            T[m + i, :, m] = w[i, :]
            T[64 + m + i, :, OH + m] = w[i, :]
    return T.reshape(128, WT_COLS)


def _make_blobs(shard, wt_host):
    """shard: (32, 64, 64) float32 -> per-chunk host blobs."""
    x_host = np.ascontiguousarray(
        shard.reshape(NPAIR, 128, W).transpose(1, 0, 2))    # (128, 16, 64)
    blobs = {}
    s = 0
    for c, pairs in enumerate(CHUNK_PAIRS):
        xc = x_host[:, s:s + pairs, :].reshape(128, pairs * W)
        s += pairs
        parts = [xc, np.zeros((128, XPAD), dtype=np.float32)]
        if c == 0:
            parts.insert(0, wt_host)
        blobs[f"blob{c}"] = np.ascontiguousarray(np.concatenate(parts, axis=1))
    return blobs


def _run(inputs, trace=False):
    from concourse.bass_utils import run_bass_kernel_spmd

    data = np.ascontiguousarray(np.asarray(inputs["data"], dtype=np.float32))
    conv_w = np.asarray(inputs["conv_w"], dtype=np.float32)
    conv_b = float(np.asarray(inputs["conv_b"], dtype=np.float32).reshape(-1)[0])

    if _COMPILED.get("key") != conv_b:
        _COMPILED["nc"] = _build_bass(conv_b)
        _COMPILED["key"] = conv_b

    wt_host = _toeplitz(conv_w)
    in_maps = [
        _make_blobs(data[c * PER_CORE:(c + 1) * PER_CORE, 0], wt_host)
        for c in range(N_CORES)
    ]

    res = run_bass_kernel_spmd(
        _COMPILED["nc"], in_maps, list(range(N_CORES)), trace=trace)

    total = np.float64(0.0)
    for c in range(N_CORES):
        total += res.results[c]["sums"].astype(np.float64).sum()
    classical_mean = total / (B * OH * OW)
    out = np.full((B,), 0.5 * classical_mean, dtype=np.float32)
    return out, res


def kernel(**inputs):
    out, _ = _run(inputs, trace=False)
    return out
